# revision 1
# baseline (speedup 1.0000x reference)
"""Trainium2 Bass kernel for nn_AlignGrapher (8 NeuronCores, SPMD).

Restructurings (validated vs reference in numpy to ~1e-6 absmax):
 - c_aggregation's 156-step sequential smoothing == fixed linear operator
   L [256,256] on patch positions, precomputed on host.
 - fc1 (1x1 conv) folded into the patchify-conv weights -> one [4096,4096]
   matmul per (branch,batch) + one [256,256] position-mix matmul.
 - conv-output [4096,256] -> image [64,128,128] is a flat reinterpret.
 - KNN: top-9 of -dist == top-9 of similarity (xn . zu); row scaling of the
   queries is ranking-irrelevant, so xn stays unnormalized.
 - EdgeConv: max_k relu(gc_w @ [x_i; x_j - x_i]) == relu(U + max_k V[:,idx])
   with U = (A-B)@xn + gc_b, V = (B/4)@z_sum  (gc_w = [A|B]).

Sharding: phase 1 data-parallel over 8 (branch, batch) units; phase 2 the
core pair (b, 4+b) splits batch b's 12544 query rows in half.  Training-mode
BatchNorm stats and the pair activation exchange go over device collectives;
x/y-role divergence is handled with 0/1 mask inputs (same SPMD program).

fp32 matmuls can carry at most one semaphore wait after walrus splits them,
so "pe_gate" dummies absorb multi-DMA dependencies into the PE clock first.
"""
import os
import numpy as np

import concourse.bass as bass
import concourse.bacc as bacc_mod
import concourse.mybir as mybir
from concourse.tile import TileContext

C = 64
P = 8
IMG = 112
KNN = 9
E = 4096          # P*P*C
NPOS = 256        # 16*16 patch grid
PN = 14
N = IMG * IMG     # 12544
M = 3136          # 56*56
HALF = N // 2     # 6272
NT = HALF // 128  # 49 query tiles per core
BN_EPS = 1e-5
NCORES = 8

F32 = mybir.dt.float32
U32 = mybir.dt.uint32
NEG_BIG = -1.0e30
PHASE = os.environ.get('KERNEL_PHASE', 'full')

# ----------------------------------------------------------------------------
# host-side constant prep
# ----------------------------------------------------------------------------

def _build_L():
    idxs = [i * PN + j for i in range(1, PN - 1) for j in range(1, PN)]
    offs = np.array([-PN, PN, -1, 1, -PN - 1, -PN + 1, PN - 1, PN + 1], np.int64)
    L = np.eye(NPOS, dtype=np.float64)
    for idx in idxs:
        L[idx, :] = L[idx + offs, :].mean(axis=0)
    return L


def _patchify(img):
    xp = np.zeros((C, IMG + 2 * P, IMG + 2 * P), dtype=np.float32)
    xp[:, P:IMG + P, P:IMG + P] = img
    return xp.reshape(C, 16, P, 16, P).transpose(0, 2, 4, 1, 3).reshape(E, NPOS).copy()


def _host_prep(inputs):
    L = _build_L()
    cagg_w = np.asarray(inputs['cagg_w'], np.float64)
    fc1_w = np.asarray(inputs['fc1_w'], np.float64)
    Wc4 = cagg_w.reshape(E, C * P * P).reshape(C, P * P, C * P * P)
    Wf = np.einsum('oc,cqk->oqk', fc1_w, Wc4).reshape(E, C * P * P)
    b4 = np.asarray(inputs['cagg_b'], np.float64).reshape(C, P * P)
    bf = (fc1_w @ b4).reshape(E) + np.repeat(np.asarray(inputs['fc1_b'], np.float64), P * P)

    gc_w = np.asarray(inputs['gc_w'], np.float32)
    A = gc_w[:, :C]; Bw = gc_w[:, C:]
    ambt = np.zeros((65, 128), np.float32)
    ambt[:64, :] = (A - Bw).T
    ambt[64, :] = np.asarray(inputs['gc_b'], np.float32)
    bq = (Bw / 4.0).T.copy()                       # [64,128]
    fc2wt = np.asarray(inputs['fc2_w'], np.float32).T.copy()   # [128,64]

    bnp = np.zeros((64, 8), np.float32)
    bnp[:, 0] = inputs['bn1_g']; bnp[:, 1] = inputs['bn1_b']
    bnp[:, 2] = inputs['bn2_g']; bnp[:, 3] = inputs['bn2_b']
    bnp[:, 4] = inputs['fc2_b']

    return {
        'wft': np.ascontiguousarray(Wf.T, dtype=np.float32),    # [4096(k), 4096(e)]
        'lt': np.ascontiguousarray(L.T, dtype=np.float32),      # [256(q), 256(p)]
        'bfv': bf.astype(np.float32).reshape(E, 1),
        'ambt': ambt, 'bq': bq, 'fc2wt': fc2wt, 'bnp': bnp,
        'ident': np.eye(128, dtype=np.float32),
    }


# ----------------------------------------------------------------------------
# device program
# ----------------------------------------------------------------------------

def build_program():
    nc = bacc_mod.Bacc('TRN2', target_bir_lowering=False, debug=False,
                       num_devices=NCORES)

    pmat_d = nc.declare_dram_parameter('pmat', [E, NPOS], F32, isOutput=False)
    wft_d = nc.declare_dram_parameter('wft', [E, E], F32, isOutput=False)
    lt_d = nc.declare_dram_parameter('lt', [NPOS, NPOS], F32, isOutput=False)
    bf_d = nc.declare_dram_parameter('bfv', [E, 1], F32, isOutput=False)
    ambt_d = nc.declare_dram_parameter('ambt', [65, 128], F32, isOutput=False)
    bq_d = nc.declare_dram_parameter('bq', [64, 128], F32, isOutput=False)
    fc2wt_d = nc.declare_dram_parameter('fc2wt', [128, 64], F32, isOutput=False)
    bnp_d = nc.declare_dram_parameter('bnp', [64, 8], F32, isOutput=False)
    ident_d = nc.declare_dram_parameter('ident', [128, 128], F32, isOutput=False)
    masks_d = nc.declare_dram_parameter('masks', [128, 2], F32, isOutput=False)
    out_d = nc.declare_dram_parameter('out_half', [64, HALF], F32, isOutput=True)

    AL = mybir.AluOpType
    AF = mybir.ActivationFunctionType
    GRP_ALL = [list(range(NCORES))]
    GRP_PAIR = [[b, b + 4] for b in range(4)]

    with TileContext(nc) as tc:
        with tc.tile_pool(name='dram', bufs=1, space='DRAM') as dram, \
             tc.tile_pool(name='const', bufs=1) as cpool, \
             tc.tile_pool(name='persist', bufs=1) as ppool:

            t2d = dram.tile([E, NPOS], F32, tag='t2d')
            cc1_in = dram.tile([64, 4], F32, tag='cc1i')
            cc1_out = dram.tile([64, 4], F32, tag='cc1o')
            cc2a_in = dram.tile([64, HALF + M], F32, tag='cc2ai')
            cc2a_out = dram.tile([64, HALF + M], F32, tag='cc2ao')
            cc2b_in = dram.tile([M, 128], F32, tag='cc2bi')
            cc2b_out = dram.tile([M, 128], F32, tag='cc2bo')
            cc3_in = dram.tile([64, 2], F32, tag='cc3i')
            cc3_out = dram.tile([64, 2], F32, tag='cc3o')
            rn_d = dram.tile([1, M], F32, tag='rnd')

            # all small constants packed into one tile (SBUF slots are 4KiB-padded)
            cst = cpool.tile([128, 1024], F32, tag='cst')
            lt_sb = cst[:, 0:512].rearrange('p (a s) -> p a s', a=2)
            ident_sb = cst[:, 512:640]
            ambt_sb = cst[:65, 640:768]
            bq_sb = cst[:64, 768:896]
            fc2wt_sb = cst[:, 896:960]
            bnp_sb = cst[:64, 960:968]
            masks_sb = cst[:, 968:970]
            bfs_sb = cst[:, 970:1002]
            ones_sb = cst[:64, 1002:1003]
            nc.sync.dma_start(out=lt_sb, in_=lt_d.rearrange('(a p) s -> p a s', p=128))
            nc.sync.dma_start(out=ident_sb, in_=ident_d[:, :])
            nc.sync.dma_start(out=ambt_sb, in_=ambt_d[:, :])
            nc.sync.dma_start(out=bq_sb, in_=bq_d[:, :])
            nc.sync.dma_start(out=fc2wt_sb, in_=fc2wt_d[:, :])
            nc.sync.dma_start(out=bnp_sb, in_=bnp_d[:, :])
            nc.sync.dma_start(out=masks_sb, in_=masks_d[:, :])
            nc.sync.dma_start(out=bfs_sb,
                              in_=bf_d.rearrange('(et p) one -> p (et one)', p=128))
            nc.vector.memset(ones_sb, 1.0)

            gate_sb = cpool.tile([128, 128], F32, tag='gate')
            gpsc = [0]

            def pe_gate(src_ap, pool=None, tag='g'):
                # Absorb pending deps of src_ap's producers into the PE clock:
                # ACT copies a slice into gate_sb, then a dummy PE matmul reads
                # gate_sb (single-wait, fp32-legal).
                nc.scalar.copy(out=gate_sb[:src_ap.shape[0], :src_ap.shape[-1]],
                               in_=src_ap)
                if pool is None:
                    with tc.tile_pool(name=f'gps{gpsc[0]}', bufs=1,
                                      space='PSUM') as gps:
                        gp = gps.tile([128, 128], F32, tag='g',
                                      name=f'gp{gpsc[0]}')
                        nc.tensor.matmul(gp, lhsT=gate_sb, rhs=gate_sb,
                                         start=True, stop=True)
                else:
                    gp = pool.tile([128, 128], F32, tag=tag, name=f'gp{gpsc[0]}')
                    nc.tensor.matmul(gp, lhsT=gate_sb, rhs=gate_sb,
                                     start=True, stop=True)
                gpsc[0] += 1

            mx64 = masks_sb[:64, 0:1]
            my64 = masks_sb[:64, 1:2]
            my128 = masks_sb[:, 1:2]

            # persistent across phase 2
            q_sb = ppool.tile([65, HALF], F32, tag='q')
            zux_sb = ppool.tile([64, M], F32, tag='zux')
            outpre_sb = ppool.tile([64, HALF], F32, tag='outpre')

            # ---------------- phase 1a+1b: folded conv + position mix ------
            with tc.tile_pool(name='t1tp', bufs=1) as t1tp:
                t1t_sb = t1tp.tile([128, 2, E], F32, tag='t1t')    # [:, qc, e]
                with tc.tile_pool(name='pm', bufs=1) as pmp, \
                     tc.tile_pool(name='wstream', bufs=3) as wsp, \
                     tc.tile_pool(name='ps1', bufs=8, space='PSUM') as ps1:
                    pmat_sb = pmp.tile([128, 32, NPOS], F32, tag='pmat')
                    nc.sync.dma_start(out=pmat_sb,
                                      in_=pmat_d.rearrange('(k p) s -> p k s', p=128))
                    # absorb const-DMA sems one at a time, then pmat
                    for gi, off in enumerate((0, 512, 640, 768, 896, 960,
                                              968, 970, 1002)):
                        nc.scalar.copy(out=gate_sb[:, gi:gi + 1],
                                       in_=cst[:, off:off + 1])
                    pe_gate(pmat_sb[:, 0, 0:128], pool=ps1, tag='acc')
                    for eh in range(2):
                        if eh == 1:
                            pe_gate(t1t_sb[:, 0, 0:128], pool=ps1, tag='acc')
                        psums = [ps1.tile([128, 512], F32, tag='acc',
                                          name=f'acc{eh}_{i}') for i in range(8)]
                        for k in range(32):
                            wt = wsp.tile([128, 2048], F32, tag='w')
                            nc.sync.dma_start(
                                out=wt, in_=wft_d[k * 128:(k + 1) * 128,
                                                 eh * 2048:(eh + 1) * 2048])
                            for qc in range(2):
                                for ec in range(4):
                                    nc.tensor.matmul(
                                        psums[qc * 4 + ec],
                                        lhsT=pmat_sb[:, k, qc * 128:(qc + 1) * 128],
                                        rhs=wt[:, ec * 512:(ec + 1) * 512],
                                        start=(k == 0), stop=(k == 31))
                        for qc in range(2):
                            for ec in range(4):
                                nc.scalar.copy(
                                    out=t1t_sb[:, qc, eh * 2048 + ec * 512:
                                               eh * 2048 + (ec + 1) * 512],
                                    in_=psums[qc * 4 + ec])

                with tc.tile_pool(name='t2p', bufs=1) as t2p, \
                     tc.tile_pool(name='ps2', bufs=4, space='PSUM') as ps2:
                    t2_sb = t2p.tile([128, 32, NPOS], F32, tag='t2')
                    for et in range(32):
                        ps = ps2.tile([128, NPOS], F32, tag='mm2')
                        for qc in range(2):
                            nc.tensor.matmul(ps,
                                             lhsT=t1t_sb[:, qc, et * 128:(et + 1) * 128],
                                             rhs=lt_sb[:, qc, :],
                                             start=(qc == 0), stop=(qc == 1))
                        nc.vector.tensor_scalar(out=t2_sb[:, et, :], in0=ps,
                                                scalar1=bfs_sb[:, et:et + 1],
                                                scalar2=None, op0=AL.add)
                    # bounce through DRAM to switch to channel-major layout
                    nc.sync.dma_start(out=t2d.rearrange('(et p) s -> p et s', p=128),
                                      in_=t2_sb)

            # f_sb: channel-major cropped image rows (cols 8:120 are valid)
            with tc.tile_pool(name='fp', bufs=1) as fp:
                f_sb = fp.tile([64, IMG, 128], F32, tag='f')
                valid = f_sb[:, :, 8:120]                          # [64,112,112]
                img_view = t2d.rearrange('(c q) (t s) -> c (q t) s', c=64, t=2)
                nc.sync.dma_start(out=f_sb, in_=img_view[:, 8:120, :])

                # ------------ phase 1c: BN1 stats + allreduce + apply ------
                with tc.tile_pool(name='bn1', bufs=1) as bnp1:
                    recs = bnp1.tile([64, IMG, 6], F32, tag='recs')
                    for i in range(IMG):
                        nc.vector.bn_stats(out=recs[:, i, :], in_=valid[:, i, :])
                    sc = bnp1.tile([64, 16], F32, tag='sc')
                    mv = sc[:, 0:2]
                    tmp = sc[:, 2:3]
                    ssum = sc[:, 3:4]
                    ssq = sc[:, 4:5]
                    su = sc[:, 5:6]
                    qu = sc[:, 6:7]
                    mean = sc[:, 7:8]
                    var = sc[:, 8:9]
                    alpha = sc[:, 9:10]
                    beta = sc[:, 10:11]
                    stage = sc[:, 12:16]
                    nc.vector.bn_aggr(out=mv, in_=recs)
                    # S = mean*N, Q = (var+mean^2)*N ; masked into 4 cols
                    nc.vector.tensor_scalar(out=ssum, in0=mv[:, 0:1], scalar1=float(N),
                                            scalar2=None, op0=AL.mult)
                    nc.vector.tensor_tensor(out=tmp, in0=mv[:, 0:1], in1=mv[:, 0:1],
                                            op=AL.mult)
                    nc.vector.tensor_tensor(out=ssq, in0=mv[:, 1:2], in1=tmp, op=AL.add)
                    nc.vector.tensor_scalar(out=ssq, in0=ssq, scalar1=float(N),
                                            scalar2=None, op0=AL.mult)
                    nc.vector.tensor_scalar(out=stage[:, 0:1], in0=ssum, scalar1=mx64,
                                            scalar2=None, op0=AL.mult)
                    nc.vector.tensor_scalar(out=stage[:, 1:2], in0=ssq, scalar1=mx64,
                                            scalar2=None, op0=AL.mult)
                    nc.vector.tensor_scalar(out=stage[:, 2:3], in0=ssum, scalar1=my64,
                                            scalar2=None, op0=AL.mult)
                    nc.vector.tensor_scalar(out=stage[:, 3:4], in0=ssq, scalar1=my64,
                                            scalar2=None, op0=AL.mult)
                    nc.sync.dma_start(out=cc1_in[:, :], in_=stage)
                    nc.gpsimd.collective_compute(
                        'AllReduce', AL.add, replica_groups=GRP_ALL,
                        ins=[cc1_in.opt()], outs=[cc1_out.opt()])
                    red = bnp1.tile([64, 4], F32, tag='red')
                    nc.sync.dma_start(out=red, in_=cc1_out[:, :])
                    nc.vector.tensor_scalar(out=tmp, in0=red[:, 2:3], scalar1=my64,
                                            scalar2=None, op0=AL.mult)
                    nc.vector.scalar_tensor_tensor(out=su, in0=red[:, 0:1], scalar=mx64,
                                                   in1=tmp, op0=AL.mult, op1=AL.add)
                    nc.vector.tensor_scalar(out=tmp, in0=red[:, 3:4], scalar1=my64,
                                            scalar2=None, op0=AL.mult)
                    nc.vector.scalar_tensor_tensor(out=qu, in0=red[:, 1:2], scalar=mx64,
                                                   in1=tmp, op0=AL.mult, op1=AL.add)
                    ncnt = 1.0 / (4.0 * N)
                    nc.vector.tensor_scalar(out=mean, in0=su, scalar1=ncnt,
                                            scalar2=None, op0=AL.mult)
                    nc.vector.tensor_scalar(out=var, in0=qu, scalar1=ncnt,
                                            scalar2=None, op0=AL.mult)
                    nc.vector.tensor_tensor(out=tmp, in0=mean, in1=mean, op=AL.mult)
                    nc.vector.tensor_tensor(out=var, in0=var, in1=tmp, op=AL.subtract)
                    nc.vector.tensor_scalar(out=var, in0=var, scalar1=BN_EPS,
                                            scalar2=None, op0=AL.add)
                    nc.scalar.activation(out=var, in_=var, func=AF.Sqrt)
                    nc.vector.reciprocal(out=var, in_=var)
                    nc.vector.tensor_tensor(out=alpha, in0=var, in1=bnp_sb[:, 0:1],
                                            op=AL.mult)
                    nc.vector.tensor_tensor(out=tmp, in0=mean, in1=alpha, op=AL.mult)
                    nc.vector.tensor_tensor(out=beta, in0=bnp_sb[:, 1:2], in1=tmp,
                                            op=AL.subtract)
                    nc.vector.tensor_scalar(out=valid, in0=valid, scalar1=alpha,
                                            scalar2=beta, op0=AL.mult, op1=AL.add)

                # ------------ phase 1d: pool, zu, Vt, exchange -------------
                with tc.tile_pool(name='pz', bufs=1) as pz, \
                     tc.tile_pool(name='psn', bufs=2, space='PSUM') as psn, \
                     tc.tile_pool(name='vtp', bufs=2) as vtp:
                    # 2x2 sum-pool: columns first into the shared staging slot
                    stg = pz.tile([64, HALF], F32, tag='stg')
                    tcol = stg.rearrange('c (h s) -> c h s', h=IMG)[:, :, :56]
                    vc = valid.rearrange('c h (s two) -> c h s two', two=2)
                    nc.vector.tensor_tensor(out=tcol, in0=vc[:, :, :, 0],
                                            in1=vc[:, :, :, 1], op=AL.add)
                    z_sb = pz.tile([64, M], F32, tag='z')
                    tr = tcol.rearrange('c (h two) s -> c h two s', two=2)
                    nc.vector.tensor_tensor(out=z_sb.rearrange('c (h s) -> c h s', h=56),
                                            in0=tr[:, :, 0, :], in1=tr[:, :, 1, :],
                                            op=AL.add)
                    # column norms via ones-matmul of z^2 (zsq in staging slot)
                    zsq = stg[:, :M]
                    nc.vector.tensor_tensor(out=zsq, in0=z_sb, in1=z_sb, op=AL.mult)
                    nsq = pz.tile([1, M], F32, tag='nsq')
                    for j in range(7):
                        psq = psn.tile([1, 448], F32, tag='nrm')
                        nc.tensor.matmul(psq, lhsT=ones_sb,
                                         rhs=zsq[:, j * 448:(j + 1) * 448],
                                         start=True, stop=True)
                        nc.scalar.copy(out=nsq[:, j * 448:(j + 1) * 448], in_=psq)
                    nc.vector.reciprocal(out=nsq, in_=nsq)
                    nc.scalar.activation(out=nsq, in_=nsq, func=AF.Sqrt)
                    # broadcast 1/||z|| across partitions via DRAM; zu in staging
                    nc.sync.dma_start(out=rn_d[:, :], in_=nsq)
                    zu = stg[:, :M]
                    nc.sync.dma_start(out=zu, in_=rn_d[:, :].to_broadcast([64, M]))
                    nc.vector.tensor_tensor(out=zu, in0=z_sb, in1=zu, op=AL.mult)
                    nc.vector.tensor_scalar(out=zu, in0=zu, scalar1=my64,
                                            scalar2=None, op0=AL.mult)
                    nc.sync.dma_start(out=cc2a_in[:, HALF:], in_=zu)

                    # Vt[mchunk, f] = z^T @ Bq, masked by my, into cc2b_in
                    for mc in range(25):
                        w = 128 if mc < 24 else 64
                        pv = psn.tile([128, 128], F32, tag='vt')
                        nc.tensor.matmul(pv[:w, :], lhsT=z_sb[:, mc * 128:mc * 128 + w],
                                         rhs=bq_sb, start=True, stop=True)
                        vt = vtp.tile([128, 128], F32, tag='vtsb')
                        nc.scalar.activation(out=vt[:w, :], in_=pv[:w, :],
                                             func=AF.Identity, scale=my128[:w, :])
                        nc.sync.dma_start(out=cc2b_in[mc * 128:mc * 128 + w, :],
                                          in_=vt[:w, :])
                    # xn-bottom-half contribution (staging slot again)
                    nc.vector.tensor_scalar(
                        out=stg.rearrange('c (h w) -> c h w', h=56),
                        in0=valid[:, 56:, :], scalar1=mx64, scalar2=None, op0=AL.mult)
                    nc.sync.dma_start(out=cc2a_in[:, :HALF], in_=stg)
                    nc.gpsimd.collective_compute(
                        'AllReduce', AL.add, replica_groups=GRP_PAIR,
                        ins=[cc2a_in.opt()], outs=[cc2a_out.opt()])
                    nc.gpsimd.collective_compute(
                        'AllReduce', AL.add, replica_groups=GRP_PAIR,
                        ins=[cc2b_in.opt()], outs=[cc2b_out.opt()])

                    # queries_aug [65, HALF]: my*exchanged + mx*local-top
                    nc.vector.memset(q_sb[64:65, :], 1.0)
                    nc.sync.dma_start(out=zux_sb, in_=cc2a_out[:, HALF:])
                    nc.sync.dma_start(out=stg, in_=cc2a_out[:, :HALF])
                    nc.vector.tensor_scalar(out=stg, in0=stg, scalar1=my64,
                                            scalar2=None, op0=AL.mult)
                    nc.vector.scalar_tensor_tensor(
                        out=q_sb[0:64, :].rearrange('c (h w) -> c h w', h=56),
                        in0=valid[:, :56, :], scalar=mx64,
                        in1=stg.rearrange('c (h w) -> c h w', h=56),
                        op0=AL.mult, op1=AL.add)

            if PHASE == '1':
                nc.sync.dma_start(out=out_d[:, :], in_=q_sb[0:64, :])

            # ---------------- phase 2: sim + top9 + edgeconv + fc2 ---------
            if PHASE != '1':
                pe_gate(zux_sb[:, 0:128])
                with tc.tile_pool(name='psim', bufs=5, space='PSUM') as psim, \
                     tc.tile_pool(name='pmisc', bufs=3, space='PSUM') as pmisc, \
                     tc.tile_pool(name='simp', bufs=2) as simp, \
                     tc.tile_pool(name='idxp', bufs=2) as idxp, \
                     tc.tile_pool(name='vgp', bufs=2) as vgp, \
                     tc.tile_pool(name='edge', bufs=2) as edgep:
                    for t in range(NT):
                        qs = q_sb[0:64, t * 128:(t + 1) * 128]
                        sim = simp.tile([128, M], F32, tag='sim')
                        for j in range(7):
                            psj = psim.tile([128, 448], F32, tag='s')
                            nc.tensor.matmul(psj, lhsT=qs,
                                             rhs=zux_sb[:, j * 448:(j + 1) * 448],
                                             start=True, stop=True)
                            nc.scalar.copy(out=sim[:, j * 448:(j + 1) * 448], in_=psj)
                        # idxt packs [mx8 | mx9 | idx8 | idx9] as 4B columns
                        idxt = idxp.tile([128, 32], F32, tag='idxt')
                        mx8 = idxt[:, 0:8]
                        mx9 = idxt[:, 8:16]
                        idx8 = idxt[:, 16:24].bitcast(U32)
                        idx9 = idxt[:, 24:32].bitcast(U32)
                        nc.vector.max(out=mx8, in_=sim)
                        nc.vector.max_index(out=idx8, in_max=mx8, in_values=sim)
                        simz = simp.tile([128, M], F32, tag='simz')
                        nc.vector.match_replace(out=simz, in_to_replace=mx8,
                                                in_values=sim, imm_value=NEG_BIG)
                        nc.vector.max(out=mx9, in_=simz)
                        nc.vector.max_index(out=idx9, in_max=mx9, in_values=simz)

                        vg = vgp.tile([128, KNN, 128], F32, tag='vg')
                        if PHASE == '2':
                            nc.vector.memset(vg, 0.0)
                        else:
                            for k in range(KNN):
                                off = idx8[:, k:k + 1] if k < 8 else idx9[:, 0:1]
                                nc.gpsimd.indirect_dma_start(
                                    out=vg[:, k, :], out_offset=None,
                                    in_=cc2b_out[:, :],
                                    in_offset=bass.IndirectOffsetOnAxis(ap=off,
                                                                        axis=0))
                        acc = vg[:, 0, :]
                        for k in range(1, KNN):
                            nc.vector.tensor_tensor(out=acc, in0=acc, in1=vg[:, k, :],
                                                    op=AL.max)

                        pu = pmisc.tile([128, 128], F32, tag='m')
                        nc.tensor.matmul(pu, lhsT=q_sb[:, t * 128:(t + 1) * 128],
                                         rhs=ambt_sb, start=True, stop=True)
                        hts = edgep.tile([128, 256], F32, tag='hts')
                        ht = hts[:, 0:128]
                        hs = hts[:, 128:256]
                        nc.vector.tensor_tensor(out=ht, in0=pu, in1=acc, op=AL.add)
                        nc.scalar.activation(out=ht, in_=ht, func=AF.Relu)
                        ph = pmisc.tile([128, 128], F32, tag='m', name=f'ph{t}')
                        nc.tensor.transpose(ph, ht, ident_sb)
                        nc.scalar.copy(out=hs, in_=ph)
                        po = pmisc.tile([64, 128], F32, tag='m', name=f'po{t}')
                        nc.tensor.matmul(po, lhsT=fc2wt_sb, rhs=hs,
                                         start=True, stop=True)
                        nc.scalar.activation(out=outpre_sb[:, t * 128:(t + 1) * 128],
                                             in_=po, func=AF.Identity,
                                             bias=bnp_sb[:, 4:5])

                # ------------ phase 3: BN2 + output ------------------------
                with tc.tile_pool(name='bn2', bufs=1) as bnp2:
                    recs2 = bnp2.tile([64, 14, 6], F32, tag='recs2')
                    opv = outpre_sb.rearrange('c (a b) -> c a b', b=448)
                    for i in range(14):
                        nc.vector.bn_stats(out=recs2[:, i, :], in_=opv[:, i, :])
                    sc2 = bnp2.tile([64, 16], F32, tag='sc2')
                    mv2 = sc2[:, 0:2]
                    st2 = sc2[:, 2:4]
                    tmp2 = sc2[:, 4:5]
                    mean2 = sc2[:, 5:6]
                    var2 = sc2[:, 6:7]
                    a2 = sc2[:, 7:8]
                    b2 = sc2[:, 8:9]
                    nc.vector.bn_aggr(out=mv2, in_=recs2)
                    nc.vector.tensor_scalar(out=st2[:, 0:1], in0=mv2[:, 0:1],
                                            scalar1=float(HALF), scalar2=None,
                                            op0=AL.mult)
                    nc.vector.tensor_tensor(out=tmp2, in0=mv2[:, 0:1],
                                            in1=mv2[:, 0:1], op=AL.mult)
                    nc.vector.tensor_tensor(out=st2[:, 1:2], in0=mv2[:, 1:2],
                                            in1=tmp2, op=AL.add)
                    nc.vector.tensor_scalar(out=st2[:, 1:2], in0=st2[:, 1:2],
                                            scalar1=float(HALF), scalar2=None,
                                            op0=AL.mult)
                    nc.sync.dma_start(out=cc3_in[:, :], in_=st2)
                    nc.gpsimd.collective_compute(
                        'AllReduce', AL.add, replica_groups=GRP_ALL,
                        ins=[cc3_in.opt()], outs=[cc3_out.opt()])
                    red2 = bnp2.tile([64, 2], F32, tag='red2')
                    nc.sync.dma_start(out=red2, in_=cc3_out[:, :])
                    ncnt2 = 1.0 / (NCORES * HALF)
                    nc.vector.tensor_scalar(out=mean2, in0=red2[:, 0:1], scalar1=ncnt2,
                                            scalar2=None, op0=AL.mult)
                    nc.vector.tensor_scalar(out=var2, in0=red2[:, 1:2], scalar1=ncnt2,
                                            scalar2=None, op0=AL.mult)
                    nc.vector.tensor_tensor(out=tmp2, in0=mean2, in1=mean2, op=AL.mult)
                    nc.vector.tensor_tensor(out=var2, in0=var2, in1=tmp2,
                                            op=AL.subtract)
                    nc.vector.tensor_scalar(out=var2, in0=var2, scalar1=BN_EPS,
                                            scalar2=None, op0=AL.add)
                    nc.scalar.activation(out=var2, in_=var2, func=AF.Sqrt)
                    nc.vector.reciprocal(out=var2, in_=var2)
                    nc.vector.tensor_tensor(out=a2, in0=var2, in1=bnp_sb[:, 2:3],
                                            op=AL.mult)
                    nc.vector.tensor_tensor(out=tmp2, in0=mean2, in1=a2, op=AL.mult)
                    nc.vector.tensor_tensor(out=b2, in0=bnp_sb[:, 3:4], in1=tmp2,
                                            op=AL.subtract)
                    nc.vector.tensor_scalar(out=outpre_sb, in0=outpre_sb, scalar1=a2,
                                            scalar2=b2, op0=AL.mult, op1=AL.add)
                    nc.sync.dma_start(out=out_d[:, :], in_=outpre_sb)

    nc.compile()
    return nc


_PROGRAM = None


def _get_program():
    global _PROGRAM
    if _PROGRAM is None:
        _PROGRAM = build_program()
    return _PROGRAM


def make_inmaps(inputs):
    prep = _host_prep(inputs)
    x = np.asarray(inputs['x'], np.float32)
    y = np.asarray(inputs['y'], np.float32)
    in_maps = []
    for core in range(NCORES):
        img = x[core] if core < 4 else y[core - 4]
        mx, my = (1.0, 0.0) if core < 4 else (0.0, 1.0)
        masks = np.zeros((128, 2), np.float32)
        masks[:, 0] = mx
        masks[:, 1] = my
        in_maps.append({
            'pmat': _patchify(img),
            'masks': masks,
            **{k: prep[k] for k in ('wft', 'lt', 'bfv', 'ambt', 'bq', 'fc2wt',
                                    'bnp', 'ident')},
        })
    return in_maps


def assemble(results, inputs):
    x = np.asarray(inputs['x'], np.float32)
    y = np.asarray(inputs['y'], np.float32)
    out = np.empty((4, 64, N), np.float32)
    for b in range(4):
        out[b, :, :HALF] = results[b]['out_half']
        out[b, :, HALF:] = results[b + 4]['out_half']
    out = out.reshape(4, 64, IMG, IMG)
    return out + x, out + y


def kernel(**inputs):
    from concourse.bass_utils import run_bass_kernel_spmd
    nc = _get_program()
    in_maps = make_inmaps(inputs)
    res = run_bass_kernel_spmd(nc, in_maps, core_ids=list(range(NCORES)))
    return assemble(res.results, inputs)

